# revision 13
# baseline (speedup 1.0000x reference)
"""Cross-attention kernel for Trainium2, sharded across 8 NeuronCores.

Sharding: batch (B=2) x query-row blocks (2048 -> 4 blocks of 512). Core c
handles batch c//4 and query rows [512*(c%4), 512*(c%4+1)). Each core runs
the FULL K/V projections (duplicated within a batch group) and all 16 heads
of attention for its query block, then the full output projection -- so each
core owns a disjoint [512, 1024] output slice and NO collective is needed.

q/k/v are shipped pre-transposed ([D, tokens]) and the weights pre-laid-out
for SBUF, so the kernel does no PE transposes. Matmul-path data is bf16;
softmax statistics and PSUM accumulation stay fp32. The attention scale
1/sqrt(64) is folded into Wq on the host.
"""

import sys

sys.path.insert(0, "/opt/trn_rl_repo")

import numpy as np

import concourse.bass as bass
import concourse.mybir as mybir
import concourse.tile as tile
from concourse import bacc

F32 = mybir.dt.float32
BF16 = mybir.dt.bfloat16

B = 2
S = 2048  # both Sq and Sk
D = 1024
NCORES = 8
H = 16  # heads (all on every core)
DH = 64
PAIRS = H // 2  # head pairs sharing a 128-partition block
QROWS = S // 4  # 512 query rows per core
TCOL = 512  # key token column width for projections
NTCOL = S // TCOL  # 4
NIC = D // 128  # 8 input-dim chunks
NKB = S // 128  # 16 key blocks
NQB = QROWS // 128  # 4 output row blocks


def build_compute(tc, ins, st, phases=7):
    """Everything per iteration. May sit inside a timing repeat loop (all
    state tiles are rewritten every pass)."""
    nc = tc.nc
    qt, kt, vt = ins["qt"], ins["kt"], ins["vt"]
    wq_sb, wk_sb, wv_sb = st["wq_sb"], st["wk_sb"], st["wv_sb"]
    wo = ins["wo"]
    bq_sb, bk_sb = st["bq_sb"], st["bk_sb"]
    QT, KT, VA, OTs = st["QT"], st["KT"], st["VA"], st["OTs"]
    dram2 = st["dram2"]
    out_ext = st["out_ext"]

    # ---- Phase 1: projections (inputs arrive pre-transposed) ----
    if phases & 1:
        with (
            tc.tile_pool(name="xin", bufs=2) as xinp,
            tc.tile_pool(name="ps_p", bufs=3, space="PSUM") as ps_p,
        ):
            # Q: one 512-token block, 8 output-dim blocks
            qsb = xinp.tile([128, NIC, QROWS], BF16, name="xsb", tag="x")
            nc.sync.dma_start(out=qsb[:], in_=qt.rearrange("(c p) t -> p c t", p=128))
            for db in range(NIC):
                pp = ps_p.tile([128, D], F32, name="pp")
                for ic in range(NIC):
                    nc.tensor.matmul(
                        pp[:, 0:QROWS],
                        wq_sb[:, ic, db * 128 : (db + 1) * 128],
                        qsb[:, ic, :],
                        start=(ic == 0),
                        stop=(ic == NIC - 1),
                    )
                nc.vector.tensor_scalar_add(
                    QT[:, db, :], pp[:, 0:QROWS], bq_sb[:, db : db + 1]
                )
            # K: 4 token blocks x 8 output-dim blocks
            for tcol in range(NTCOL):
                ksb = xinp.tile([128, NIC, TCOL], BF16, name="xsb", tag="x")
                nc.sync.dma_start(
                    out=ksb[:],
                    in_=kt[:, tcol * TCOL : (tcol + 1) * TCOL].rearrange(
                        "(c p) t -> p c t", p=128
                    ),
                )
                for db in range(NIC):
                    pp = ps_p.tile([128, D], F32, name="pp")
                    for ic in range(NIC):
                        nc.tensor.matmul(
                            pp[:, 0:TCOL],
                            wk_sb[:, ic, db * 128 : (db + 1) * 128],
                            ksb[:, ic, :],
                            start=(ic == 0),
                            stop=(ic == NIC - 1),
                        )
                    nc.vector.tensor_scalar_add(
                        KT[:, db, tcol * TCOL : (tcol + 1) * TCOL],
                        pp[:, 0:TCOL],
                        bk_sb[:, db : db + 1],
                    )
            # V: natural [token, dim] output, written per head into VA
            # (dims at cols 0:64, ones col at 64).
            for tcol in range(NTCOL):
                vsb = xinp.tile([128, NIC, TCOL], BF16, name="xsb", tag="x")
                nc.sync.dma_start(
                    out=vsb[:],
                    in_=vt[:, tcol * TCOL : (tcol + 1) * TCOL].rearrange(
                        "(c p) t -> p c t", p=128
                    ),
                )
                for tb in range(4):
                    pp = ps_p.tile([128, D], F32, name="pp")
                    for ic in range(NIC):
                        for vh in range(2):
                            nc.tensor.matmul(
                                pp[:, vh * TCOL : (vh + 1) * TCOL],
                                vsb[:, ic, tb * 128 : (tb + 1) * 128],
                                wv_sb[:, ic, vh * TCOL : (vh + 1) * TCOL],
                                start=(ic == 0),
                                stop=(ic == NIC - 1),
                            )
                    kb = tcol * 4 + tb
                    nc.vector.tensor_copy(
                        VA[:, kb, :, 0:DH],
                        pp[:].rearrange("p (h d) -> p h d", d=DH),
                    )

    # ---- Phase 2: attention, head pairs on PE row groups ----
    # Pair p = heads (2p, 2p+1): head 2p on partitions 0:64 of dim block p,
    # head 2p+1 on partitions 64:128. Pairs processed two at a time (pi=0,1)
    # so the Exp activations run on full [128, 1024] tiles.
    if phases & 2:
        with (
            tc.tile_pool(name="ps_S", bufs=2, space="PSUM") as psS,
            tc.tile_pool(name="ps_O", bufs=1, space="PSUM") as psO,
            tc.tile_pool(name="PT", bufs=6) as PTp,
            tc.tile_pool(name="rb", bufs=2) as rbp,
            tc.tile_pool(name="dn", bufs=2) as dnp,
        ):
            for g in range(4):  # pair-group: pairs (2g, 2g+1)
                pO = {
                    (pi, hh): psO.tile(
                        [128, QROWS], F32, name=f"pO{pi}{hh}", tag=f"pO{pi}{hh}"
                    )
                    for pi in range(2)
                    for hh in range(2)
                }
                for kb in range(NKB):
                    pss = [psS.tile([128, 1024], F32, name="ps") for _ in range(2)]
                    for pi in range(2):
                        p = 2 * g + pi
                        for hh in range(2):
                            nc.tensor.matmul(
                                pss[hh][:, pi * QROWS : (pi + 1) * QROWS],
                                KT[hh * 64 : hh * 64 + 64, p,
                                   kb * 128 : (kb + 1) * 128],
                                QT[hh * 64 : hh * 64 + 64, p, :],
                                start=True,
                                stop=True,
                            )
                    pts = []
                    for hh in range(2):
                        pt = PTp.tile([128, 1024], BF16, name="pt")
                        nc.scalar.activation(
                            pt[:], pss[hh][:], mybir.ActivationFunctionType.Exp
                        )
                        pts.append(pt)
                    for pi in range(2):
                        p = 2 * g + pi
                        for hh in range(2):
                            h = 2 * p + hh
                            nc.tensor.matmul(
                                pO[(pi, hh)][0 : DH + 1, :],
                                VA[:, kb, h, :],
                                pts[hh][:, pi * QROWS : (pi + 1) * QROWS],
                                start=(kb == 0),
                                stop=(kb == NKB - 1),
                            )
                # normalize all four heads of the group
                for pi in range(2):
                    p = 2 * g + pi
                    for hh in range(2):
                        h = 2 * p + hh
                        dn = dnp.tile(
                            [65, QROWS], F32, name=f"dn{pi}{hh}", tag=f"dn{pi}{hh}"
                        )
                        nc.vector.reciprocal(
                            dn[64:65, :], pO[(pi, hh)][64:65, :]
                        )
                        scr = dram2.tile([1, QROWS], F32, name="scr", tag="scr")
                        nc.sync.dma_start(out=scr[:], in_=dn[64:65, :])
                        rb = rbp.tile(
                            [64, QROWS], F32, name=f"rb{pi}{hh}", tag=f"rb{pi}{hh}"
                        )
                        scr_ap = scr[:]
                        bcast = bass.AP(
                            tensor=scr_ap.tensor,
                            offset=scr_ap.offset,
                            ap=[[0, 64], [1, QROWS]],
                        )
                        nc.sync.dma_start(out=rb[:], in_=bcast)
                        nc.vector.tensor_mul(
                            OTs[:, h, :], pO[(pi, hh)][0:DH, :], rb[:]
                        )

    # ---- Phase 3: output projection (heads summed in PSUM, full Wo) ----
    if phases & 4:
        with (
            tc.tile_pool(name="ps_Z", bufs=2, space="PSUM") as psZ,
            tc.tile_pool(name="zsb", bufs=3) as zp,
            tc.tile_pool(name="wop", bufs=1) as wop,
        ):
            wo_sb = wop.tile([64, H, D], BF16)
            nc.sync.dma_start(
                out=wo_sb[:], in_=wo.rearrange("p (c n) -> p c n", c=H)
            )
            for qb in range(NQB):
                zz = [
                    psZ.tile([128, TCOL], F32, name=f"z{n2}", tag=f"z{n2}")
                    for n2 in range(2)
                ]
                for h in range(H):
                    for n2 in range(2):
                        nc.tensor.matmul(
                            zz[n2][:],
                            OTs[:, h, qb * 128 : (qb + 1) * 128],
                            wo_sb[:, h, n2 * TCOL : (n2 + 1) * TCOL],
                            start=(h == 0),
                            stop=(h == H - 1),
                        )
                zt = zp.tile([128, D], F32)
                for n2 in range(2):
                    nc.vector.tensor_copy(zt[:, n2 * TCOL : (n2 + 1) * TCOL], zz[n2][:])
                nc.sync.dma_start(
                    out=out_ext[qb * 128 : (qb + 1) * 128, :], in_=zt[:]
                )


def build_attention_kernel(tc, es, ins, out_ext, loop_n=1, rs_n=1, phases=7):
    nc = tc.nc
    wq, wk, wv, wo = ins["wq"], ins["wk"], ins["wv"], ins["wo"]
    bq, bk = ins["bq"], ins["bk"]

    wpool = es.enter_context(tc.tile_pool(name="wpool", bufs=1))
    big = es.enter_context(tc.tile_pool(name="big", bufs=1))
    dram2 = es.enter_context(tc.tile_pool(name="dram2", bufs=2, space="DRAM"))

    # Weights into SBUF (host pre-laid-out: straight copies).
    wq_sb = wpool.tile([128, NIC, D], BF16)
    wk_sb = wpool.tile([128, NIC, D], BF16)
    wv_sb = wpool.tile([128, NIC, D], BF16)
    nc.sync.dma_start(out=wq_sb[:], in_=wq.rearrange("p (c n) -> p c n", c=NIC))
    nc.sync.dma_start(out=wk_sb[:], in_=wk.rearrange("p (c n) -> p c n", c=NIC))
    nc.sync.dma_start(out=wv_sb[:], in_=wv.rearrange("p (c n) -> p c n", c=NIC))
    bq_sb = wpool.tile([128, NIC], F32)
    bk_sb = wpool.tile([128, NIC], F32)
    nc.sync.dma_start(out=bq_sb[:], in_=bq)
    nc.sync.dma_start(out=bk_sb[:], in_=bk)

    # Persistent activations.
    QT = big.tile([128, NIC, QROWS], BF16)  # [dim%128, dimblock, qtok]
    KT = big.tile([128, NIC, S], BF16)
    VA = big.tile([128, NKB, H, DH + 1], BF16)  # V + ones col per head
    OTs = big.tile([64, H, QROWS], BF16)  # normalized O^T per head
    nc.vector.memset(VA[:, :, :, DH : DH + 1], 1.0)

    st = dict(
        wq_sb=wq_sb, wk_sb=wk_sb, wv_sb=wv_sb,
        bq_sb=bq_sb, bk_sb=bk_sb, QT=QT, KT=KT, VA=VA, OTs=OTs,
        dram2=dram2, out_ext=out_ext,
    )

    if loop_n > 1:
        with tc.For_i(0, loop_n, 1):
            build_compute(tc, ins, st, phases=phases)
    else:
        build_compute(tc, ins, st, phases=phases)


def build_nc(loop_n=1, rs_n=1, phases=7):
    nc = bacc.Bacc(
        "TRN2", target_bir_lowering=False, debug=False, num_devices=NCORES
    )
    ins = {}
    ins["qt"] = nc.dram_tensor("qt", [D, QROWS], BF16, kind="ExternalInput").ap()
    ins["kt"] = nc.dram_tensor("kt", [D, S], BF16, kind="ExternalInput").ap()
    ins["vt"] = nc.dram_tensor("vt", [D, S], BF16, kind="ExternalInput").ap()
    for nm in ("wq", "wk", "wv"):
        ins[nm] = nc.dram_tensor(nm, [128, NIC * D], BF16, kind="ExternalInput").ap()
    ins["wo"] = nc.dram_tensor("wo", [64, H * D], BF16, kind="ExternalInput").ap()
    ins["bq"] = nc.dram_tensor("bq", [128, NIC], F32, kind="ExternalInput").ap()
    ins["bk"] = nc.dram_tensor("bk", [128, NIC], F32, kind="ExternalInput").ap()
    out_ext = nc.dram_tensor("out", [QROWS, D], F32, kind="ExternalOutput").ap()

    from contextlib import ExitStack

    with tile.TileContext(nc) as tc:
        with ExitStack() as es:
            build_attention_kernel(
                tc, es, ins, out_ext, loop_n=loop_n, rs_n=rs_n, phases=phases
            )
    nc.compile()
    return nc


def make_in_maps(q, k, v, Wq, bq, Wk, bk, Wv, bv, Wo, bo):
    """Host-side sharding: transpose activations, pre-lay-out weights for
    SBUF, fold the attention scale into Wq, cast matmul-path data to bf16."""
    import ml_dtypes

    bf16 = ml_dtypes.bfloat16
    scale = DH**-0.5
    kT = [np.ascontiguousarray(k[b].T).astype(bf16) for b in range(B)]
    vT = [np.ascontiguousarray(v[b].T).astype(bf16) for b in range(B)]
    qT = [
        [
            np.ascontiguousarray(q[b, r * QROWS : (r + 1) * QROWS, :].T).astype(bf16)
            for r in range(4)
        ]
        for b in range(B)
    ]
    # wq_sb[p, ic, n] = Wq[ic*128 + p, n]
    def lay_w(W):
        return np.ascontiguousarray(
            W.reshape(NIC, 128, D).transpose(1, 0, 2).reshape(128, NIC * D)
        ).astype(bf16)

    wq_l = lay_w(Wq * scale)
    wk_l = lay_w(Wk)
    wv_l = lay_w(Wv)
    # wo_sb[j, h, n] = Wo[h*64 + j, n]
    wo_l = np.ascontiguousarray(
        Wo.reshape(H, DH, D).transpose(1, 0, 2).reshape(DH, H * D)
    ).astype(bf16)
    bq_l = np.ascontiguousarray(
        (bq * scale).reshape(NIC, 128).T, dtype=np.float32
    )
    bk_l = np.ascontiguousarray(bk.reshape(NIC, 128).T, dtype=np.float32)
    in_maps = []
    for c in range(NCORES):
        b, r = c // 4, c % 4
        in_maps.append(
            {
                "qt": qT[b][r],
                "kt": kT[b],
                "vt": vT[b],
                "wq": wq_l,
                "wk": wk_l,
                "wv": wv_l,
                "wo": wo_l,
                "bq": bq_l,
                "bk": bk_l,
            }
        )
    return in_maps


def assemble_output(results, bv, bo, Wo):
    out = np.empty((B, S, D), np.float32)
    for c in range(NCORES):
        b, r = c // 4, c % 4
        out[b, r * QROWS : (r + 1) * QROWS, :] = results[c]["out"]
    # bv's contribution commutes through softmax-normalized attention and the
    # output projection as a constant row offset; bo is a plain offset.
    bo_eff = np.asarray(bo, np.float64) + np.asarray(bv, np.float64) @ np.asarray(
        Wo, np.float64
    )
    if np.any(bo_eff):
        out += bo_eff[None, None, :].astype(np.float32)
    return out


_NC_CACHE = None


def kernel(q, k, v, Wq, bq, Wk, bk, Wv, bv, Wo, bo):
    global _NC_CACHE
    from concourse.bass_utils import run_bass_kernel_spmd

    args = [
        np.asarray(x, np.float32) for x in (q, k, v, Wq, bq, Wk, bk, Wv, bv, Wo, bo)
    ]
    q, k, v, Wq, bq, Wk, bk, Wv, bv, Wo, bo = args
    if _NC_CACHE is None:
        _NC_CACHE = build_nc()
    nc = _NC_CACHE
    in_maps = make_in_maps(q, k, v, Wq, bq, Wk, bk, Wv, bv, Wo, bo)
    res = run_bass_kernel_spmd(nc, in_maps, core_ids=list(range(NCORES)))
    return assemble_output(res.results, bv, bo, Wo)
